# revision 1
# baseline (speedup 1.0000x reference)
"""CropRandomizer (pos_enc=True) Trainium2 kernel.

Full inputs: images [64,3,240,240] f32, crop_inds_h/w [64,8] i32 (0..23).
Full output: [512, 5, 216, 216] f32 (3 img channels + 2 pos channels, 8
random 216x216 crops per image).

Strategy (data-parallel over 8 NeuronCores, 8 images per core):
- Host prepends the two positional-encoding planes (constant meshgrid) to
  each image -> per-core src [8, 5, 240, 240].
- On device, each plane (image b, channel c) is staged in SBUF twice, split
  into two overlapping 132-row segments: seg0 = rows 0..131 on partition
  24c + 12s + b (s=0), seg1 = rows 108..239 (s=1).  With this layout any
  216-row crop window [h0, h0+216) (h0 <= 24) decomposes into rows
  [h0, h0+108) of seg0 and the same local rows of seg1, so one crop is a
  single 3-dim DMA: partitions [b : b+109 : 12] (10 partitions = (c,s)
  pairs, s fastest), free dims [ds(h0,108), ds(w0,216)].  The destination
  (the output crop) is fully contiguous.
- h0/w0 are loaded from SBUF into sequencer registers at runtime
  (values_load) so one compiled program serves all cores / any offsets.
"""

import numpy as np

import concourse.bacc as bacc
import concourse.bass as bass
import concourse.mybir as mybir
import concourse.tile as tile
from concourse.bass import ds
from concourse.bass_utils import run_bass_kernel_spmd

# Dynamic (register) SBUF AP offsets are lowered as raw linear addresses in
# the 64-bit SBUF map, where consecutive partitions are 256KB (= 65536 f32
# elements) apart — HW-verified by probing.  Static offsets/dim-steps use
# tensor-flat units, so a dynamic AP must carry its partition base in
# hardware units instead.
SBUF_PART_STRIDE_ELEMS = 65536

H = W = 240
CROP = 216
TOP_ROWS = 132          # seg0: rows 0..131
BOT_ROW0 = 108          # seg1: rows 108..239
SEG_ROWS = 108          # rows per crop piece
B_PER_CORE = 8
N_CROPS = 8
CP = 5                  # 3 image channels + 2 pos channels
N_CORES = 8
MAX_OFF = H - CROP - 1  # 23

_PROGRAM = None


def _build_program(repeat=1):
    nc = bacc.Bacc(
        "TRN2", target_bir_lowering=False, debug=False, enable_asserts=False
    )
    src = nc.dram_tensor(
        "src", [B_PER_CORE, CP, H, W], mybir.dt.float32, kind="ExternalInput"
    ).ap()
    ih = nc.dram_tensor(
        "ih", [1, B_PER_CORE * N_CROPS], mybir.dt.int32, kind="ExternalInput"
    ).ap()
    iw = nc.dram_tensor(
        "iw", [1, B_PER_CORE * N_CROPS], mybir.dt.int32, kind="ExternalInput"
    ).ap()
    out = nc.dram_tensor(
        "out",
        [B_PER_CORE * N_CROPS, CP, CROP, CROP],
        mybir.dt.float32,
        kind="ExternalOutput",
    ).ap()

    with tile.TileContext(nc) as tc:
        with tc.tile_pool(name="pool", bufs=1) as pool:
            planes = pool.tile([128, TOP_ROWS, W], mybir.dt.float32)
            ih_t = pool.tile([1, B_PER_CORE * N_CROPS], mybir.dt.int32)
            iw_t = pool.tile([1, B_PER_CORE * N_CROPS], mybir.dt.int32)

            nc.sync.dma_start(ih_t[:], ih[:])
            nc.sync.dma_start(iw_t[:], iw[:])

            # Stage planes: per image b, seg0 partitions {24c+b}, seg1 {24c+12+b}.
            for b in range(B_PER_CORE):
                e0, e1 = (nc.sync, nc.scalar) if b % 2 == 0 else (nc.scalar, nc.sync)
                e0.dma_start(planes[b:b + 97:24, :, :], src[b, :, 0:TOP_ROWS, :])
                e1.dma_start(
                    planes[b + 12:b + 12 + 97:24, :, :], src[b, :, BOT_ROW0:H, :]
                )

            # Crops: iterate n outer / b inner so consecutive in-flight DMAs
            # hit different partition groups (different SBUF ports).
            for j in range(B_PER_CORE * N_CROPS * repeat):
                j = j % (B_PER_CORE * N_CROPS)
                n, b = divmod(j, B_PER_CORE)
                k = b * N_CROPS + n
                eng, dma_eng = (
                    (mybir.EngineType.SP, nc.sync)
                    if j % 2 == 0
                    else (mybir.EngineType.Activation, nc.scalar)
                )
                h0 = nc.values_load(
                    ih_t[0:1, k:k + 1], engines=(eng,),
                    min_val=0, max_val=MAX_OFF, skip_runtime_bounds_check=True,
                )
                w0 = nc.values_load(
                    iw_t[0:1, k:k + 1], engines=(eng,),
                    min_val=0, max_val=MAX_OFF, skip_runtime_bounds_check=True,
                )
                base = planes[0:109:12, ds(h0, SEG_ROWS), ds(w0, CROP)]
                src_ap = bass.AP(
                    tensor=base.tensor,
                    offset=base.offset + b * SBUF_PART_STRIDE_ELEMS,
                    ap=base.ap,
                )
                dma_eng.dma_start(
                    out[k].rearrange("c (s r) w -> (c s) r w", s=2), src_ap
                )

    nc.compile()
    return nc


def _get_program():
    global _PROGRAM
    if _PROGRAM is None:
        _PROGRAM = _build_program()
    return _PROGRAM


def _pos_planes():
    yy, xx = np.meshgrid(
        np.arange(H, dtype=np.float32) / H,
        np.arange(W, dtype=np.float32) / W,
        indexing="ij",
    )
    return np.stack((yy, xx))  # [2, H, W]


def make_in_maps(images, crop_inds_h, crop_inds_w):
    pos = np.broadcast_to(_pos_planes()[None], (B_PER_CORE, 2, H, W))
    in_maps = []
    for c in range(N_CORES):
        sl = slice(c * B_PER_CORE, (c + 1) * B_PER_CORE)
        src = np.ascontiguousarray(
            np.concatenate(
                (np.asarray(images[sl], dtype=np.float32), pos), axis=1
            )
        )
        in_maps.append(
            {
                "src": src,
                "ih": np.ascontiguousarray(
                    np.asarray(crop_inds_h[sl], dtype=np.int32).reshape(1, -1)
                ),
                "iw": np.ascontiguousarray(
                    np.asarray(crop_inds_w[sl], dtype=np.int32).reshape(1, -1)
                ),
            }
        )
    return in_maps


def kernel(images, crop_inds_h, crop_inds_w):
    nc = _get_program()
    in_maps = make_in_maps(images, crop_inds_h, crop_inds_w)
    res = run_bass_kernel_spmd(nc, in_maps, core_ids=list(range(N_CORES)))
    return np.concatenate([r["out"] for r in res.results], axis=0)



# revision 3
# speedup vs baseline: 1.1547x; 1.1547x over previous
"""CropRandomizer (pos_enc=True) Trainium2 kernel.

Full inputs: images [64,3,240,240] f32, crop_inds_h/w [64,8] i32 (0..23).
Full output: [512, 5, 216, 216] f32 (3 img channels + 2 pos channels, 8
random 216x216 crops per image).

Strategy (data-parallel over 8 NeuronCores, 8 images per core):
- Host prepends the two positional-encoding planes (constant meshgrid) to
  each image -> per-core src [8, 5, 240, 240] in DRAM.
- Each crop is a single DRAM->DRAM DMA: out[k] (contiguous [5,216,216])
  <- src[b, :, h0:h0+216, w0:w0+216] (rows of 864B, row stride 960B).
  No SBUF staging: DMA engines read the strided window from HBM and write
  the contiguous crop back, so the only DMA payload is the output itself
  (59.7MB/core) plus the 512B offset table.
- h0/w0 are loaded from SBUF into sequencer registers at runtime
  (values_load) so one compiled program serves all cores / any offsets.
  Crop DMAs alternate between the SP and Activation HWDGE queues so
  descriptor generation pipelines ahead of the serial DMA transfers.
"""

import numpy as np

import concourse.bacc as bacc
import concourse.bass as bass
import concourse.mybir as mybir
import concourse.tile as tile
from concourse.bass import ds
from concourse.bass_utils import run_bass_kernel_spmd

H = W = 240
CROP = 216
B_PER_CORE = 8
N_CROPS = 8
CP = 5                  # 3 image channels + 2 pos channels
N_CORES = 8
MAX_OFF = H - CROP - 1  # 23

_PROGRAM = None


def _build_program():
    nc = bacc.Bacc(
        "TRN2", target_bir_lowering=False, debug=False, enable_asserts=False
    )
    src = nc.dram_tensor(
        "src", [B_PER_CORE, CP, H, W], mybir.dt.float32, kind="ExternalInput"
    ).ap()
    ihw = nc.dram_tensor(
        "ihw", [1, 2 * B_PER_CORE * N_CROPS], mybir.dt.int32, kind="ExternalInput"
    ).ap()
    out = nc.dram_tensor(
        "out",
        [B_PER_CORE * N_CROPS, CP, CROP, CROP],
        mybir.dt.float32,
        kind="ExternalOutput",
    ).ap()

    with tile.TileContext(nc) as tc:
        with tc.tile_pool(name="pool", bufs=1) as pool:
            ihw_t = pool.tile([1, 2 * B_PER_CORE * N_CROPS], mybir.dt.int32)
            nc.sync.dma_start(ihw_t[:], ihw[:])

            for k in range(B_PER_CORE * N_CROPS):
                b = k // N_CROPS
                eng, dma_eng = (
                    (mybir.EngineType.SP, nc.sync)
                    if k % 2 == 0
                    else (mybir.EngineType.Activation, nc.scalar)
                )
                _, (h0, w0) = nc.values_load_multi_w_load_instructions(
                    ihw_t[0:1, 2 * k:2 * k + 2], engines=(eng,),
                    min_val=0, max_val=MAX_OFF, skip_runtime_bounds_check=True,
                )
                dma_eng.dma_start(out[k], src[b, :, ds(h0, CROP), ds(w0, CROP)])

    nc.compile()
    return nc


def _get_program():
    global _PROGRAM
    if _PROGRAM is None:
        _PROGRAM = _build_program()
    return _PROGRAM


def _pos_planes():
    yy, xx = np.meshgrid(
        np.arange(H, dtype=np.float32) / H,
        np.arange(W, dtype=np.float32) / W,
        indexing="ij",
    )
    return np.stack((yy, xx))  # [2, H, W]


def make_in_maps(images, crop_inds_h, crop_inds_w):
    pos = np.broadcast_to(_pos_planes()[None], (B_PER_CORE, 2, H, W))
    in_maps = []
    for c in range(N_CORES):
        sl = slice(c * B_PER_CORE, (c + 1) * B_PER_CORE)
        src = np.ascontiguousarray(
            np.concatenate(
                (np.asarray(images[sl], dtype=np.float32), pos), axis=1
            )
        )
        ihw = np.stack(
            (
                np.asarray(crop_inds_h[sl], dtype=np.int32).reshape(-1),
                np.asarray(crop_inds_w[sl], dtype=np.int32).reshape(-1),
            ),
            axis=1,
        ).reshape(1, -1)
        in_maps.append({"src": src, "ihw": np.ascontiguousarray(ihw)})
    return in_maps


def kernel(images, crop_inds_h, crop_inds_w):
    nc = _get_program()
    in_maps = make_in_maps(images, crop_inds_h, crop_inds_w)
    res = run_bass_kernel_spmd(nc, in_maps, core_ids=list(range(N_CORES)))
    return np.concatenate([r["out"] for r in res.results], axis=0)


# revision 4
# speedup vs baseline: 1.1735x; 1.0163x over previous
"""CropRandomizer (pos_enc=True) Trainium2 kernel.

Full inputs: images [64,3,240,240] f32, crop_inds_h/w [64,8] i32 (0..23).
Full output: [512, 5, 216, 216] f32 (3 img channels + 2 pos channels, 8
random 216x216 crops per image).

Strategy (data-parallel over 8 NeuronCores, 8 images per core):
- Host prepends the two positional-encoding planes (constant meshgrid) to
  each image -> per-core src [8, 5, 240, 240] in DRAM.
- Each crop is a single DRAM->DRAM DMA: out[k] (contiguous [5,216,216])
  <- src[b, :, h0:h0+216, w0:w0+216] (rows of 864B, row stride 960B).
  No SBUF staging: DMA engines read the strided window from HBM and write
  the contiguous crop back, so the only DMA payload is the output itself
  (59.7MB/core) plus the 512B offset table.
- h0/w0 are loaded from SBUF into sequencer registers at runtime
  (values_load) so one compiled program serves all cores / any offsets.
  Crop DMAs alternate between the SP and Activation HWDGE queues so
  descriptor generation pipelines ahead of the serial DMA transfers.
"""

import numpy as np

import concourse.bacc as bacc
import concourse.bass as bass
import concourse.mybir as mybir
import concourse.tile as tile
from concourse.bass import ds
from concourse.bass_utils import run_bass_kernel_spmd

H = W = 240
CROP = 216
B_PER_CORE = 8
N_CROPS = 8
CP = 5                  # 3 image channels + 2 pos channels
N_CORES = 8
MAX_OFF = H - CROP - 1  # 23

_PROGRAM = None


def _build_program():
    nc = bacc.Bacc(
        "TRN2", target_bir_lowering=False, debug=False, enable_asserts=False
    )
    src = nc.dram_tensor(
        "src", [B_PER_CORE, CP, H, W], mybir.dt.float32, kind="ExternalInput"
    ).ap()
    ihw = nc.dram_tensor(
        "ihw", [1, 2 * B_PER_CORE * N_CROPS], mybir.dt.int32, kind="ExternalInput"
    ).ap()
    out = nc.dram_tensor(
        "out",
        [B_PER_CORE * N_CROPS, CP, CROP, CROP],
        mybir.dt.float32,
        kind="ExternalOutput",
    ).ap()

    with tile.TileContext(nc) as tc:
        for k in range(B_PER_CORE * N_CROPS):
            b = k // N_CROPS
            eng, dma_eng = (
                (mybir.EngineType.SP, nc.sync)
                if k % 2 == 0
                else (mybir.EngineType.Activation, nc.scalar)
            )
            # Sequencer register loads read the offset pair straight from the
            # DRAM table — no SBUF staging DMA on the critical path.
            _, (h0, w0) = nc.values_load_multi_w_load_instructions(
                ihw[0:1, 2 * k:2 * k + 2], engines=(eng,),
                min_val=0, max_val=MAX_OFF, skip_runtime_bounds_check=True,
            )
            dma_eng.dma_start(out[k], src[b, :, ds(h0, CROP), ds(w0, CROP)])

    nc.compile()
    return nc


def _get_program():
    global _PROGRAM
    if _PROGRAM is None:
        _PROGRAM = _build_program()
    return _PROGRAM


def _pos_planes():
    yy, xx = np.meshgrid(
        np.arange(H, dtype=np.float32) / H,
        np.arange(W, dtype=np.float32) / W,
        indexing="ij",
    )
    return np.stack((yy, xx))  # [2, H, W]


def make_in_maps(images, crop_inds_h, crop_inds_w):
    pos = np.broadcast_to(_pos_planes()[None], (B_PER_CORE, 2, H, W))
    in_maps = []
    for c in range(N_CORES):
        sl = slice(c * B_PER_CORE, (c + 1) * B_PER_CORE)
        src = np.ascontiguousarray(
            np.concatenate(
                (np.asarray(images[sl], dtype=np.float32), pos), axis=1
            )
        )
        ihw = np.stack(
            (
                np.asarray(crop_inds_h[sl], dtype=np.int32).reshape(-1),
                np.asarray(crop_inds_w[sl], dtype=np.int32).reshape(-1),
            ),
            axis=1,
        ).reshape(1, -1)
        in_maps.append({"src": src, "ihw": np.ascontiguousarray(ihw)})
    return in_maps


def kernel(images, crop_inds_h, crop_inds_w):
    nc = _get_program()
    in_maps = make_in_maps(images, crop_inds_h, crop_inds_w)
    res = run_bass_kernel_spmd(nc, in_maps, core_ids=list(range(N_CORES)))
    return np.concatenate([r["out"] for r in res.results], axis=0)
